# revision 26
# baseline (speedup 1.0000x reference)
"""Trainium2 Bass kernel for DeepseekV4 HCA compressor (single-shot window compression).

Computation per 128-token window:
    kv   = h @ w_kv            [128, 128]
    gate = h @ w_gate + bias   [128, 128]
    w    = softmax(gate, axis=tokens)   (per output channel)
    comp = sum(w * kv, axis=tokens)     [128]
then RMS-norm over channels and interleaved RoPE on the last 64 channels.

Sharding: 128 windows (2 batches x 64) split across 8 cores, 16 windows each.

v4 design (cost-model-driven; see plan.py for the stream/PE co-schedule):
- ALL bf16 data (position bias, w_gate, w_kv, pre-transposed h) lives in ONE
  per-core DRAM blob whose columns are in exact PE consumption order:
  [bias | per-quartet wg/h0/wkv interleave | h groups 1-4]. The DMA stream is
  36 contiguous 256KB column pieces at full bus rate.
- PE program: 5 bias identity-matmuls (one per group's gate PSUM) + 5 dummy
  matmuls form the front cushion -- they depend only on piece 0, keeping the
  PE continuously busy through the stream catch-up so no idle->busy
  transition ever resets the p-state ramp; every real matmul is visited at
  full clock and runs back-to-back with ZERO stalls (planner-verified).
- Groups of [4,4,4,3,1] windows; quartet order gq0 kq0 gq1 kq1 gq2 kq2 gq3
  kq3 (the per-group softmax overflow lands on idle DVE/ACT time, not PE).
  The last 1-window group runs all gates first so its exp/recip chain hides
  entirely under its kv quartets.
- Softmax per group: one ACT Exp per window reading the gate PSUM directly
  (bias was pre-loaded into PSUM by the identity matmul) with fused row-sum
  accum (denominator), DVE reciprocal; after kv closes: e*kv multiply,
  per-window numerator reduce, comp = num * rden.
- Hidden per-group epilogue prep: sqc = comp^2, then comp *= wn_col (the rms
  weight as a per-partition column, 1.0 on rope channels -- their weight is
  folded into the host cs tables).
- Tail (window 15 only): mul+reduce+comp, sq/wn, ONE transpose [128,16] ->
  [16,128], ones-matmul ssq row [1,16], 5-op newton fast-rsqrt (fused
  (z*z)*x tensor_scalar), rinv transpose to a per-partition column, scale,
  3-op RoPE, one output DMA.
"""

import sys

if "/opt/trn_rl_repo" not in sys.path:
    sys.path.insert(0, "/opt/trn_rl_repo")

import ml_dtypes
import numpy as np

import concourse.bacc as bacc
import concourse.mybir as mybir
import concourse.tile as tile
from concourse.bass_utils import run_bass_kernel_spmd
from concourse.masks import make_identity

# Problem shapes (hardcoded per contest contract)
B, S, H = 2, 8192, 2048
M = 128          # compress rate (window length)
D = 128          # head dim
T = S // M       # 64 windows per batch
NCORES = 8
WPC = (B * T) // NCORES   # 16 windows per core
GWS = [4, 4, 4, 3, 1]     # windows per group (uneven: small tail group)
GOFF = [0, 4, 8, 12, 15]  # window offset of each group
NG = len(GWS)
KC = H // 128             # 16 contraction chunks
ROPE_DIM = 64
HALF = ROPE_DIM // 2
THETA = 10000.0
EPS = 1e-6
NDUMMY = 6                # front cushion dummies (see plan.py)

F32 = mybir.dt.float32
BF16 = mybir.dt.bfloat16
I32 = mybir.dt.int32
AF = mybir.ActivationFunctionType
ALU = mybir.AluOpType

NP_BF16 = ml_dtypes.bfloat16

BIAS_COLS = 512  # one tiled bias block [128, 512]; groups use [:, :GM]


# ---- blob column layout (shared by host packing and device views) ----
def _blob_layout():
    segs = [("bias", BIAS_COLS)]
    gm0 = GWS[0] * M
    for q in range(4):
        for k in range(4 * q, 4 * q + 4):
            segs.append((f"wg{k}", D))
        for k in range(4 * q, 4 * q + 4):
            segs.append((f"h0_{k}", gm0))
        for k in range(4 * q, 4 * q + 4):
            segs.append((f"wkv{k}", D))
    for g in range(1, NG):
        for k in range(KC):
            segs.append((f"h{g}_{k}", GWS[g] * M))
    off = {}
    cur = 0
    for name, cols in segs:
        off[name] = cur
        cur += cols
    return off, cur


BLOB_OFF, BLOB_COLS = _blob_layout()


def _pieces():
    """Planner-tuned piece cuts: first piece 1536c (bias+wg q0+h0_0), then
    uniform 1024c pieces aligned to segment boundaries."""
    bounds = sorted(set(BLOB_OFF.values()) | {BLOB_COLS})
    first = BLOB_OFF["h0_0"] + GWS[0] * M
    cuts = [0, first]
    for b in bounds:
        if b <= first:
            continue
        if b - cuts[-1] >= 1024 or b == BLOB_COLS:
            cuts.append(b)
    if cuts[-1] != BLOB_COLS:
        cuts.append(BLOB_COLS)
    return [(cuts[i], cuts[i + 1]) for i in range(len(cuts) - 1)]


PIECES = _pieces()


def _build_nc():
    nc = bacc.Bacc(None, target_bir_lowering=False)

    blob_in = nc.dram_tensor("blob_in", [128, BLOB_COLS], BF16, kind="ExternalInput")
    # cos/sin table (128 cols) + rms weight rows for the nope half (64 cols)
    cswn_in = nc.dram_tensor("cswn_in", [128, 2 * ROPE_DIM + ROPE_DIM], F32, kind="ExternalInput")
    out_d = nc.dram_tensor("out_d", [WPC, D], F32, kind="ExternalOutput")

    with tile.TileContext(nc) as tc:
        with (
            tc.tile_pool(name="constp", bufs=1) as constp,
            tc.tile_pool(name="epp", bufs=2) as epp,
            tc.tile_pool(name="smallp", bufs=2) as smallp,
            tc.tile_pool(name="gtp", bufs=NG, space="PSUM") as gtp,
            tc.tile_pool(name="kvp", bufs=2, space="PSUM") as kvp,
            tc.tile_pool(name="ctp", bufs=1, space="PSUM") as ctp,
            tc.tile_pool(name="finalp", bufs=1) as finalp,
        ):
            # --- constants / epilogue state ---
            ident = constp.tile([128, 128], F32, name="ident")
            make_identity(nc, ident)
            ident_bf = constp.tile([128, 128], BF16, name="ident_bf")
            nc.vector.tensor_copy(ident_bf[:, :], ident[:, :])

            comp = constp.tile([D, WPC], F32, name="comp")
            num = constp.tile([D, WPC], F32, name="num")
            out_sb = finalp.tile([128, D], F32, name="out_sb")
            zc = constp.tile([128, 1], F32, name="zc")
            nc.vector.memset(zc[:, :], 0.0)
            # preload the exp ACT table while the first DMAs run
            warm = constp.tile([128, 1], F32, name="warm")
            nc.scalar.activation(warm[:, :], zc[:, :], AF.Exp, bias=zc[:, :])

            blob_sb = constp.tile([128, BLOB_COLS], BF16, name="blob_sb")
            cswn_sb = constp.tile([128, 2 * ROPE_DIM + ROPE_DIM], F32, name="cswn_sb")
            cs_sb = cswn_sb[:, : 2 * ROPE_DIM]
            wn_sb = cswn_sb[:, 2 * ROPE_DIM :]

            def seg(name, cols):
                o = BLOB_OFF[name]
                return blob_sb[:, o : o + cols]

            bias_sb = seg("bias", BIAS_COLS)

            # --- DMA stream: contiguous blob pieces in consumption order ---
            for i, (a, b) in enumerate(PIECES):
                nc.sync.dma_start(out=blob_sb[:, a:b], in_=blob_in[:, a:b])
                if i == 8:
                    # small f32 piece; needed by the first kv_back (~12us) --
                    # the +183ns shift to later h pieces is absorbed by the
                    # wire lead in groups 1-4
                    nc.sync.dma_start(out=cswn_sb, in_=cswn_in[:, :])

            # --- PE front cushion: bias identity-matmuls + dummies ---
            gts = [
                gtp.tile([D, GWS[g] * M], F32, name=f"gt{g}", tag="gt")
                for g in range(NG)
            ]
            # instruction order tuned for the p-state visit rule: the first
            # three PE instructions are visited before t=3us (mid clock), so
            # they are the smallest ones (128-col bias + two 64-col dummies)
            scratch = ctp.tile([128, 512], F32, name="scratch", tag="ct")
            nc.tensor.matmul(
                gts[4][:, :], ident_bf[:, :], bias_sb[:, : GWS[4] * M],
                start=True, stop=False, skip_group_check=True,
            )
            for i in range(2):
                nc.tensor.matmul(
                    scratch[:, :64], ident_bf[:, :], bias_sb[:, :64],
                    start=True, stop=True, skip_group_check=True,
                )
            for g in (3, 0, 1, 2):
                nc.tensor.matmul(
                    gts[g][:, :], ident_bf[:, :], bias_sb[:, : GWS[g] * M],
                    start=True, stop=False, skip_group_check=True,
                )
            for i in range(NDUMMY):
                nc.tensor.matmul(
                    scratch[:, :], ident_bf[:, :], bias_sb[:, :512],
                    start=True, stop=True, skip_group_check=True,
                )

            # --- per-group softmax front (after gate PSUM closes) ---
            def gate_front(g, gt_ps, ep, den, rden):
                gw = GWS[g]
                gm = gw * M
                nc.scalar.activation(
                    ep[:, :gm], gt_ps[:, :], AF.Exp, bias=zc[:D, :]
                )
                nc.vector.tensor_reduce(
                    den[:, :gw],
                    ep[:, :gm].rearrange("p (w m) -> p w m", w=gw),
                    axis=mybir.AxisListType.X,
                    op=ALU.add,
                )
                nc.vector.reciprocal(rden[:, :gw], den[:, :gw])

            # --- per-group numerator + comp ---
            def kv_back(g, kv_ps, ep, rden):
                gw = GWS[g]
                gm = gw * M
                w0 = GOFF[g]
                cg = comp[:, w0 : w0 + gw]
                nc.vector.tensor_mul(ep[:, :gm], ep[:, :gm], kv_ps[:, :])
                if gw == 1:
                    # ep was pre-scaled by rden (hidden), so the reduce
                    # writes comp directly -- shortest possible tail chain
                    nc.vector.tensor_reduce(
                        cg,
                        ep[:, :gm].rearrange("p (w m) -> p w m", w=gw),
                        axis=mybir.AxisListType.X,
                        op=ALU.add,
                    )
                    return
                nc.vector.tensor_reduce(
                    num[:, w0 : w0 + gw],
                    ep[:, :gm].rearrange("p (w m) -> p w m", w=gw),
                    axis=mybir.AxisListType.X,
                    op=ALU.add,
                )
                nc.vector.tensor_mul(cg, num[:, w0 : w0 + gw], rden[:, :gw])

            # --- matmul stream ---
            for g in range(NG):
                gm = GWS[g] * M
                gt_ps = gts[g]
                kv_ps = kvp.tile([D, gm], F32, name=f"kv{g}", tag="kv")
                ep = epp.tile([D, 512], F32, name="ep", tag="ep")
                den = smallp.tile([D, 4], F32, name="den", tag="den")
                rden = smallp.tile([D, 4], F32, name="rden", tag="rden")

                def gate_q(q):
                    for k in range(4 * q, 4 * q + 4):
                        nc.tensor.matmul(
                            gt_ps[:, :],
                            seg(f"wg{k}", D),
                            seg(f"h{g}_{k}", gm),
                            start=False,
                            stop=(k == KC - 1),
                            skip_group_check=True,
                        )

                def kv_q(q):
                    for k in range(4 * q, 4 * q + 4):
                        nc.tensor.matmul(
                            kv_ps[:, :],
                            seg(f"wkv{k}", D),
                            seg(f"h{g}_{k}", gm),
                            start=(k == 0),
                            stop=(k == KC - 1),
                            skip_group_check=True,
                        )

                if g < NG - 2:
                    gate_q(0)
                    kv_q(0)
                    gate_q(1)
                    kv_q(1)
                    gate_q(2)
                    kv_q(2)
                    gate_q(3)
                    gate_front(g, gt_ps, ep, den, rden)
                    kv_q(3)
                elif g == NG - 2:
                    # close the gate two quartets early: the exp/den/recip
                    # chain must clear DVE before the last group's tail
                    gate_q(0)
                    kv_q(0)
                    gate_q(1)
                    kv_q(1)
                    gate_q(2)
                    gate_q(3)
                    gate_front(g, gt_ps, ep, den, rden)
                    kv_q(2)
                    kv_q(3)
                else:
                    gate_q(0)
                    gate_q(1)
                    gate_q(2)
                    gate_q(3)
                    gate_front(g, gt_ps, ep, den, rden)
                    # 1-window group: rden is a per-partition scalar; fold it
                    # into ep now (hides under the kv quartets)
                    nc.vector.tensor_scalar_mul(
                        ep[:, :gm], ep[:, :gm], rden[:, :1]
                    )
                    kv_q(0)
                    kv_q(1)
                    kv_q(2)
                    kv_q(3)
                kv_back(g, kv_ps, ep, rden)

            # --- tail ---
            # two transposes into two PSUM tiles: the ACT Square chain and
            # the DVE rope chain each get a private copy of comp^T (Tile
            # serializes multiple readers of one PSUM tile in emission order)
            W = WPC
            ct16 = ctp.tile([WPC, D], F32, name="ct16", tag="ct")
            ct16b = kvp.tile([WPC, D], F32, name="ct16b", tag="kv")
            nc.tensor.transpose(ct16[:, :], comp[:, :], ident[:, :])
            # ssq via ACT Square with fused row-sum (reads ct16 PSUM; the
            # 1/sqrt(D) scale makes the accum equal ssq/D directly), then a
            # 5-op newton fast-rsqrt with fused (z*z)*x tensor_scalar.
            ssq = finalp.tile([128, 1], F32, name="ssq")
            sqs = finalp.tile([128, D], F32, name="sqs")
            nc.scalar.activation(
                sqs[:W, :], ct16[:, :],
                AF.Square, bias=zc[:W, :],
                scale=float(1.0 / np.sqrt(D)),
                accum_out=ssq[:W, :],
            )
            nc.tensor.transpose(ct16b[:, :], comp[:, :], ident[:, :])
            # wn + RoPE on the UNSCALED ct16 (rinv is a per-window scalar,
            # so it commutes with the per-channel wn/cos/sin multiplies);
            # this whole DVE chain runs in parallel with the ACT Square, and
            # only one tensor_scalar rinv-multiply follows the newton.
            u2 = finalp.tile([128, D], F32, name="u2")
            t1 = finalp.tile([128, ROPE_DIM], F32, name="t1")
            t2 = finalp.tile([128, ROPE_DIM], F32, name="t2")
            nc.vector.tensor_mul(
                u2[:W, : D - ROPE_DIM],
                ct16b[:, : D - ROPE_DIM],
                wn_sb[:W, : D - ROPE_DIM],
            )
            nc.vector.tensor_mul(
                t1[:W, :], ct16b[:, D - ROPE_DIM : D], cs_sb[:W, 0:ROPE_DIM]
            )
            nc.vector.tensor_mul(
                t2[:W, 0:HALF], ct16b[:, D - HALF : D],
                cs_sb[:W, ROPE_DIM : ROPE_DIM + HALF],
            )
            nc.vector.tensor_mul(
                t2[:W, HALF:ROPE_DIM], ct16b[:, D - ROPE_DIM : D - HALF],
                cs_sb[:W, ROPE_DIM + HALF : 2 * ROPE_DIM],
            )
            nc.vector.tensor_add(
                u2[:W, D - ROPE_DIM : D], t1[:W, :], t2[:W, :]
            )
            rinv = finalp.tile([128, 1], F32, name="rinv")
            ntc = finalp.tile([128, 1], F32, name="ntc")
            vvg = ssq[:W, :]
            rig = rinv[:W, :]
            ntg = ntc[:W, :]
            nc.vector.tensor_scalar(
                out=rig.bitcast(I32), in0=vvg.bitcast(I32),
                scalar1=1, scalar2=-1,
                op0=ALU.arith_shift_right, op1=ALU.bitwise_xor,
            )
            nc.vector.tensor_scalar(
                out=rig.bitcast(I32), in0=rig.bitcast(I32),
                scalar1=0x5F3759DF + 1, scalar2=None, op0=ALU.add,
            )
            # nt = (z*z)*ssq in one fused tensor_scalar (both scalars are APs)
            nc.vector.tensor_scalar(
                out=ntg, in0=rig, scalar1=rig, scalar2=vvg,
                op0=ALU.mult, op1=ALU.mult,
            )
            nc.vector.tensor_scalar(
                out=ntg, in0=ntg,
                scalar1=-0.5, scalar2=1.5, op0=ALU.mult, op1=ALU.add,
            )
            # final newton step fused into the output scale:
            # og = (u2 * z) * nt  ==  u2 * rinv
            og = out_sb[:W, :]
            nc.vector.tensor_scalar(
                out=og, in0=u2[:W, :], scalar1=rig, scalar2=ntg,
                op0=ALU.mult, op1=ALU.mult,
            )
            nc.sync.dma_start(out=out_d[:, :], in_=og)

    nc.compile()
    return nc


_NC_CACHE = {}


def _get_nc():
    if "nc" not in _NC_CACHE:
        _NC_CACHE["nc"] = _build_nc()
    return _NC_CACHE["nc"]


def _make_in_maps(hidden_states, w_kv, w_gate, position_bias, kv_norm_weight):
    hidden_states = np.asarray(hidden_states, dtype=np.float32)
    w_kv = np.asarray(w_kv, dtype=np.float32)
    w_gate = np.asarray(w_gate, dtype=np.float32)
    position_bias = np.asarray(position_bias, dtype=np.float32)
    kv_norm_weight = np.asarray(kv_norm_weight, dtype=np.float32)

    h_flat = hidden_states.reshape(B * S, H)
    wg_chunks = w_gate.reshape(KC, 128, D).astype(NP_BF16)
    wkv_chunks = w_kv.reshape(KC, 128, D).astype(NP_BF16)
    bias4 = np.tile(position_bias.T, (1, BIAS_COLS // M)).astype(NP_BF16)
    wn = np.broadcast_to(
        kv_norm_weight[None, : D - ROPE_DIM], (128, D - ROPE_DIM)
    ).astype(np.float32)

    inv_freq = (1.0 / (THETA ** (np.arange(HALF, dtype=np.float32) / HALF))).astype(
        np.float32
    )
    in_maps = []
    for c in range(NCORES):
        hT = np.ascontiguousarray(
            h_flat[c * WPC * M : (c + 1) * WPC * M].T
        ).astype(NP_BF16)  # [H, WPC*M]
        hT3 = hT.reshape(KC, 128, WPC * M)  # [kc, p, t]

        blob = np.empty((128, BLOB_COLS), NP_BF16)
        for name, o in BLOB_OFF.items():
            if name == "bias":
                blob[:, o : o + BIAS_COLS] = bias4
            elif name.startswith("wg"):
                k = int(name[2:])
                blob[:, o : o + D] = wg_chunks[k]
            elif name.startswith("wkv"):
                k = int(name[3:])
                blob[:, o : o + D] = wkv_chunks[k]
            else:  # h<g>_<k>
                g, k = name[1:].split("_")
                g, k = int(g), int(k)
                t0 = GOFF[g] * M
                gm = GWS[g] * M
                blob[:, o : o + gm] = hT3[k, :, t0 : t0 + gm]

        t_global = (c % (T // WPC)) * WPC + np.arange(WPC, dtype=np.float32)
        pos = (t_global * M).astype(np.float32)
        freqs = pos[:, None] * inv_freq[None, :]
        cos2 = np.repeat(np.cos(freqs), 2, axis=1).astype(np.float32)
        sin2 = np.repeat(np.sin(freqs), 2, axis=1).astype(np.float32)
        kw = kv_norm_weight
        cos2 = cos2 * kw[None, D - ROPE_DIM : D]
        sinf = np.concatenate(
            [
                -sin2[:, :HALF] * kw[None, D - HALF : D],
                sin2[:, HALF:] * kw[None, D - ROPE_DIM : D - HALF],
            ],
            axis=1,
        )
        cs16 = np.concatenate([cos2, sinf], axis=1)  # [16, 128]
        cs = np.zeros((128, 2 * ROPE_DIM), np.float32)
        cs[:WPC] = cs16
        cswn = np.ascontiguousarray(np.concatenate([cs, wn], axis=1))
        in_maps.append({"blob_in": blob, "cswn_in": cswn})
    return in_maps


def _assemble(results):
    full = np.concatenate([r["out_d"] for r in results], axis=0)  # [128, 128]
    return full.reshape(B, 1, T, D).astype(np.float32)


def _run(inputs, trace=False, **spmd_kwargs):
    nc = _get_nc()
    in_maps = _make_in_maps(
        inputs["hidden_states"],
        inputs["w_kv"],
        inputs["w_gate"],
        inputs["position_bias"],
        inputs["kv_norm_weight"],
    )
    res = run_bass_kernel_spmd(
        nc, in_maps, core_ids=list(range(NCORES)), trace=trace, **spmd_kwargs
    )
    return _assemble(res.results), res


def kernel(
    hidden_states,
    q_residual=None,
    position_ids=None,
    w_kv=None,
    w_gate=None,
    position_bias=None,
    kv_norm_weight=None,
):
    out, _ = _run(
        {
            "hidden_states": hidden_states,
            "w_kv": w_kv,
            "w_gate": w_gate,
            "position_bias": position_bias,
            "kv_norm_weight": kv_norm_weight,
        }
    )
    return out


# revision 27
# speedup vs baseline: 1.0070x; 1.0070x over previous
"""Trainium2 Bass kernel for DeepseekV4 HCA compressor (single-shot window compression).

Computation per 128-token window:
    kv   = h @ w_kv            [128, 128]
    gate = h @ w_gate + bias   [128, 128]
    w    = softmax(gate, axis=tokens)   (per output channel)
    comp = sum(w * kv, axis=tokens)     [128]
then RMS-norm over channels and interleaved RoPE on the last 64 channels.

Sharding: 128 windows (2 batches x 64) split across 8 cores, 16 windows each.

v4 design (cost-model-driven; see plan.py for the stream/PE co-schedule):
- ALL bf16 data (position bias, w_gate, w_kv, pre-transposed h) lives in ONE
  per-core DRAM blob whose columns are in exact PE consumption order:
  [bias | per-quartet wg/h0/wkv interleave | h groups 1-4]. The DMA stream is
  36 contiguous 256KB column pieces at full bus rate.
- PE program: 5 bias identity-matmuls (one per group's gate PSUM) + 5 dummy
  matmuls form the front cushion -- they depend only on piece 0, keeping the
  PE continuously busy through the stream catch-up so no idle->busy
  transition ever resets the p-state ramp; every real matmul is visited at
  full clock and runs back-to-back with ZERO stalls (planner-verified).
- Groups of [4,4,4,3,1] windows; quartet order gq0 kq0 gq1 kq1 gq2 kq2 gq3
  kq3 (the per-group softmax overflow lands on idle DVE/ACT time, not PE).
  The last 1-window group runs all gates first so its exp/recip chain hides
  entirely under its kv quartets.
- Softmax per group: one ACT Exp per window reading the gate PSUM directly
  (bias was pre-loaded into PSUM by the identity matmul) with fused row-sum
  accum (denominator), DVE reciprocal; after kv closes: e*kv multiply,
  per-window numerator reduce, comp = num * rden.
- Hidden per-group epilogue prep: sqc = comp^2, then comp *= wn_col (the rms
  weight as a per-partition column, 1.0 on rope channels -- their weight is
  folded into the host cs tables).
- Tail (window 15 only): mul+reduce+comp, sq/wn, ONE transpose [128,16] ->
  [16,128], ones-matmul ssq row [1,16], 5-op newton fast-rsqrt (fused
  (z*z)*x tensor_scalar), rinv transpose to a per-partition column, scale,
  3-op RoPE, one output DMA.
"""

import sys

if "/opt/trn_rl_repo" not in sys.path:
    sys.path.insert(0, "/opt/trn_rl_repo")

import ml_dtypes
import numpy as np

import concourse.bacc as bacc
import concourse.mybir as mybir
import concourse.tile as tile
from concourse.bass_utils import run_bass_kernel_spmd
from concourse.masks import make_identity

# Problem shapes (hardcoded per contest contract)
B, S, H = 2, 8192, 2048
M = 128          # compress rate (window length)
D = 128          # head dim
T = S // M       # 64 windows per batch
NCORES = 8
WPC = (B * T) // NCORES   # 16 windows per core
GWS = [4, 4, 4, 3, 1]     # windows per group (uneven: small tail group)
GOFF = [0, 4, 8, 12, 15]  # window offset of each group
NG = len(GWS)
KC = H // 128             # 16 contraction chunks
ROPE_DIM = 64
HALF = ROPE_DIM // 2
THETA = 10000.0
EPS = 1e-6
NDUMMY = 7                # front cushion dummies (see plan.py)

F32 = mybir.dt.float32
BF16 = mybir.dt.bfloat16
I32 = mybir.dt.int32
AF = mybir.ActivationFunctionType
ALU = mybir.AluOpType

NP_BF16 = ml_dtypes.bfloat16

BIAS_COLS = 512  # one tiled bias block [128, 512]; groups use [:, :GM]


# ---- blob column layout (shared by host packing and device views) ----
def _blob_layout():
    segs = [("bias", BIAS_COLS)]
    gm0 = GWS[0] * M
    for q in range(4):
        for k in range(4 * q, 4 * q + 4):
            segs.append((f"wg{k}", D))
        for k in range(4 * q, 4 * q + 4):
            segs.append((f"h0_{k}", gm0))
        for k in range(4 * q, 4 * q + 4):
            segs.append((f"wkv{k}", D))
    for g in range(1, NG):
        for k in range(KC):
            segs.append((f"h{g}_{k}", GWS[g] * M))
    off = {}
    cur = 0
    for name, cols in segs:
        off[name] = cur
        cur += cols
    return off, cur


BLOB_OFF, BLOB_COLS = _blob_layout()


def _pieces():
    """Planner-tuned piece cuts: first piece 1536c (bias+wg q0+h0_0), then
    uniform 1024c pieces aligned to segment boundaries."""
    bounds = sorted(set(BLOB_OFF.values()) | {BLOB_COLS})
    first = BLOB_OFF["h0_0"] + GWS[0] * M
    cuts = [0, first]
    for b in bounds:
        if b <= first:
            continue
        if b - cuts[-1] >= 1024 or b == BLOB_COLS:
            cuts.append(b)
    if cuts[-1] != BLOB_COLS:
        cuts.append(BLOB_COLS)
    return [(cuts[i], cuts[i + 1]) for i in range(len(cuts) - 1)]


PIECES = _pieces()


def _build_nc():
    nc = bacc.Bacc(None, target_bir_lowering=False)

    blob_in = nc.dram_tensor("blob_in", [128, BLOB_COLS], BF16, kind="ExternalInput")
    # cos/sin table (128 cols) + rms weight rows for the nope half (64 cols)
    cswn_in = nc.dram_tensor("cswn_in", [128, 2 * ROPE_DIM + ROPE_DIM], F32, kind="ExternalInput")
    out_d = nc.dram_tensor("out_d", [WPC, D], F32, kind="ExternalOutput")

    with tile.TileContext(nc) as tc:
        with (
            tc.tile_pool(name="constp", bufs=1) as constp,
            tc.tile_pool(name="epp", bufs=2) as epp,
            tc.tile_pool(name="smallp", bufs=2) as smallp,
            tc.tile_pool(name="gtp", bufs=NG, space="PSUM") as gtp,
            tc.tile_pool(name="kvp", bufs=2, space="PSUM") as kvp,
            tc.tile_pool(name="ctp", bufs=1, space="PSUM") as ctp,
            tc.tile_pool(name="finalp", bufs=1) as finalp,
        ):
            # --- constants / epilogue state ---
            ident = constp.tile([128, 128], F32, name="ident")
            make_identity(nc, ident)
            ident_bf = constp.tile([128, 128], BF16, name="ident_bf")
            nc.vector.tensor_copy(ident_bf[:, :], ident[:, :])

            comp = constp.tile([D, WPC], F32, name="comp")
            num = constp.tile([D, WPC], F32, name="num")
            out_sb = finalp.tile([128, D], F32, name="out_sb")
            zc = constp.tile([128, 1], F32, name="zc")
            nc.vector.memset(zc[:, :], 0.0)
            # preload the exp ACT table while the first DMAs run
            warm = constp.tile([128, 1], F32, name="warm")
            nc.scalar.activation(warm[:, :], zc[:, :], AF.Exp, bias=zc[:, :])

            blob_sb = constp.tile([128, BLOB_COLS], BF16, name="blob_sb")
            cswn_sb = constp.tile([128, 2 * ROPE_DIM + ROPE_DIM], F32, name="cswn_sb")
            cs_sb = cswn_sb[:, : 2 * ROPE_DIM]
            wn_sb = cswn_sb[:, 2 * ROPE_DIM :]

            def seg(name, cols):
                o = BLOB_OFF[name]
                return blob_sb[:, o : o + cols]

            bias_sb = seg("bias", BIAS_COLS)

            # --- DMA stream: contiguous blob pieces in consumption order ---
            for i, (a, b) in enumerate(PIECES):
                nc.sync.dma_start(out=blob_sb[:, a:b], in_=blob_in[:, a:b])
                if i == 8:
                    # small f32 piece; needed by the first kv_back (~12us) --
                    # the +183ns shift to later h pieces is absorbed by the
                    # wire lead in groups 1-4
                    nc.sync.dma_start(out=cswn_sb, in_=cswn_in[:, :])

            # --- PE front cushion: bias identity-matmuls + dummies ---
            gts = [
                gtp.tile([D, GWS[g] * M], F32, name=f"gt{g}", tag="gt")
                for g in range(NG)
            ]
            # instruction order tuned for the p-state visit rule: the first
            # three PE instructions are visited before t=3us (mid clock), so
            # they are the smallest ones (128-col bias + two 64-col dummies)
            scratch = ctp.tile([128, 512], F32, name="scratch", tag="ct")
            nc.tensor.matmul(
                gts[4][:, :], ident_bf[:, :], bias_sb[:, : GWS[4] * M],
                start=True, stop=False, skip_group_check=True,
            )
            for i in range(2):
                nc.tensor.matmul(
                    scratch[:, :64], ident_bf[:, :], bias_sb[:, :64],
                    start=True, stop=True, skip_group_check=True,
                )
            for g in (3, 0, 1, 2):
                nc.tensor.matmul(
                    gts[g][:, :], ident_bf[:, :], bias_sb[:, : GWS[g] * M],
                    start=True, stop=False, skip_group_check=True,
                )
            for i in range(NDUMMY):
                nc.tensor.matmul(
                    scratch[:, :], ident_bf[:, :], bias_sb[:, :512],
                    start=True, stop=True, skip_group_check=True,
                )

            # --- per-group softmax front (after gate PSUM closes) ---
            def gate_front(g, gt_ps, ep, den, rden):
                gw = GWS[g]
                gm = gw * M
                nc.scalar.activation(
                    ep[:, :gm], gt_ps[:, :], AF.Exp, bias=zc[:D, :]
                )
                nc.vector.tensor_reduce(
                    den[:, :gw],
                    ep[:, :gm].rearrange("p (w m) -> p w m", w=gw),
                    axis=mybir.AxisListType.X,
                    op=ALU.add,
                )
                nc.vector.reciprocal(rden[:, :gw], den[:, :gw])

            # --- per-group numerator + comp ---
            def kv_back(g, kv_ps, ep, rden):
                gw = GWS[g]
                gm = gw * M
                w0 = GOFF[g]
                cg = comp[:, w0 : w0 + gw]
                nc.vector.tensor_mul(ep[:, :gm], ep[:, :gm], kv_ps[:, :])
                nc.vector.tensor_reduce(
                    num[:, w0 : w0 + gw],
                    ep[:, :gm].rearrange("p (w m) -> p w m", w=gw),
                    axis=mybir.AxisListType.X,
                    op=ALU.add,
                )
                nc.vector.tensor_mul(cg, num[:, w0 : w0 + gw], rden[:, :gw])

            # --- matmul stream ---
            for g in range(NG):
                gm = GWS[g] * M
                gt_ps = gts[g]
                kv_ps = kvp.tile([D, gm], F32, name=f"kv{g}", tag="kv")
                ep = epp.tile([D, 512], F32, name="ep", tag="ep")
                den = smallp.tile([D, 4], F32, name="den", tag="den")
                rden = smallp.tile([D, 4], F32, name="rden", tag="rden")

                def gate_q(q):
                    for k in range(4 * q, 4 * q + 4):
                        nc.tensor.matmul(
                            gt_ps[:, :],
                            seg(f"wg{k}", D),
                            seg(f"h{g}_{k}", gm),
                            start=False,
                            stop=(k == KC - 1),
                            skip_group_check=True,
                        )

                def kv_q(q):
                    for k in range(4 * q, 4 * q + 4):
                        nc.tensor.matmul(
                            kv_ps[:, :],
                            seg(f"wkv{k}", D),
                            seg(f"h{g}_{k}", gm),
                            start=(k == 0),
                            stop=(k == KC - 1),
                            skip_group_check=True,
                        )

                if g < NG - 2:
                    gate_q(0)
                    kv_q(0)
                    gate_q(1)
                    kv_q(1)
                    gate_q(2)
                    kv_q(2)
                    gate_q(3)
                    gate_front(g, gt_ps, ep, den, rden)
                    kv_q(3)
                elif g == NG - 2:
                    # close the gate two quartets early: the exp/den/recip
                    # chain must clear DVE before the last group's tail
                    gate_q(0)
                    kv_q(0)
                    gate_q(1)
                    kv_q(1)
                    gate_q(2)
                    gate_q(3)
                    gate_front(g, gt_ps, ep, den, rden)
                    kv_q(2)
                    kv_q(3)
                else:
                    gate_q(0)
                    gate_q(1)
                    gate_q(2)
                    gate_q(3)
                    gate_front(g, gt_ps, ep, den, rden)
                    kv_q(0)
                    kv_q(1)
                    kv_q(2)
                    kv_q(3)
                kv_back(g, kv_ps, ep, rden)

            # --- tail ---
            # two transposes into two PSUM tiles: the ACT Square chain and
            # the DVE rope chain each get a private copy of comp^T (Tile
            # serializes multiple readers of one PSUM tile in emission order)
            W = WPC
            ct16 = ctp.tile([WPC, D], F32, name="ct16", tag="ct")
            ct16b = kvp.tile([WPC, D], F32, name="ct16b", tag="kv")
            nc.tensor.transpose(ct16[:, :], comp[:, :], ident[:, :])
            # ssq via ACT Square with fused row-sum (reads ct16 PSUM; the
            # 1/sqrt(D) scale makes the accum equal ssq/D directly), then a
            # 5-op newton fast-rsqrt with fused (z*z)*x tensor_scalar.
            ssq = finalp.tile([128, 1], F32, name="ssq")
            sqs = finalp.tile([128, D], F32, name="sqs")
            nc.scalar.activation(
                sqs[:W, :], ct16[:, :],
                AF.Square, bias=zc[:W, :],
                scale=float(1.0 / np.sqrt(D)),
                accum_out=ssq[:W, :],
            )
            nc.tensor.transpose(ct16b[:, :], comp[:, :], ident[:, :])
            # wn + RoPE on the UNSCALED ct16 (rinv is a per-window scalar,
            # so it commutes with the per-channel wn/cos/sin multiplies);
            # this whole DVE chain runs in parallel with the ACT Square, and
            # only one tensor_scalar rinv-multiply follows the newton.
            u2 = finalp.tile([128, D], F32, name="u2")
            t1 = finalp.tile([128, ROPE_DIM], F32, name="t1")
            t2 = finalp.tile([128, ROPE_DIM], F32, name="t2")
            nc.vector.tensor_mul(
                u2[:W, : D - ROPE_DIM],
                ct16b[:, : D - ROPE_DIM],
                wn_sb[:W, : D - ROPE_DIM],
            )
            nc.vector.tensor_mul(
                t1[:W, :], ct16b[:, D - ROPE_DIM : D], cs_sb[:W, 0:ROPE_DIM]
            )
            nc.vector.tensor_mul(
                t2[:W, 0:HALF], ct16b[:, D - HALF : D],
                cs_sb[:W, ROPE_DIM : ROPE_DIM + HALF],
            )
            nc.vector.tensor_mul(
                t2[:W, HALF:ROPE_DIM], ct16b[:, D - ROPE_DIM : D - HALF],
                cs_sb[:W, ROPE_DIM + HALF : 2 * ROPE_DIM],
            )
            nc.vector.tensor_add(
                u2[:W, D - ROPE_DIM : D], t1[:W, :], t2[:W, :]
            )
            rinv = finalp.tile([128, 1], F32, name="rinv")
            ntc = finalp.tile([128, 1], F32, name="ntc")
            vvg = ssq[:W, :]
            rig = rinv[:W, :]
            ntg = ntc[:W, :]
            nc.vector.tensor_scalar(
                out=rig.bitcast(I32), in0=vvg.bitcast(I32),
                scalar1=1, scalar2=-1,
                op0=ALU.arith_shift_right, op1=ALU.bitwise_xor,
            )
            nc.vector.tensor_scalar(
                out=rig.bitcast(I32), in0=rig.bitcast(I32),
                scalar1=0x5F3759DF + 1, scalar2=None, op0=ALU.add,
            )
            # nt = (z*z)*ssq in one fused tensor_scalar (both scalars are APs)
            nc.vector.tensor_scalar(
                out=ntg, in0=rig, scalar1=rig, scalar2=vvg,
                op0=ALU.mult, op1=ALU.mult,
            )
            nc.vector.tensor_scalar(
                out=ntg, in0=ntg,
                scalar1=-0.5, scalar2=1.5, op0=ALU.mult, op1=ALU.add,
            )
            # final newton step fused into the output scale:
            # og = (u2 * z) * nt  ==  u2 * rinv
            og = out_sb[:W, :]
            nc.vector.tensor_scalar(
                out=og, in0=u2[:W, :], scalar1=rig, scalar2=ntg,
                op0=ALU.mult, op1=ALU.mult,
            )
            nc.sync.dma_start(out=out_d[:, :], in_=og)

    nc.compile()
    return nc


_NC_CACHE = {}


def _get_nc():
    if "nc" not in _NC_CACHE:
        _NC_CACHE["nc"] = _build_nc()
    return _NC_CACHE["nc"]


def _make_in_maps(hidden_states, w_kv, w_gate, position_bias, kv_norm_weight):
    hidden_states = np.asarray(hidden_states, dtype=np.float32)
    w_kv = np.asarray(w_kv, dtype=np.float32)
    w_gate = np.asarray(w_gate, dtype=np.float32)
    position_bias = np.asarray(position_bias, dtype=np.float32)
    kv_norm_weight = np.asarray(kv_norm_weight, dtype=np.float32)

    h_flat = hidden_states.reshape(B * S, H)
    wg_chunks = w_gate.reshape(KC, 128, D).astype(NP_BF16)
    wkv_chunks = w_kv.reshape(KC, 128, D).astype(NP_BF16)
    bias4 = np.tile(position_bias.T, (1, BIAS_COLS // M)).astype(NP_BF16)
    wn = np.broadcast_to(
        kv_norm_weight[None, : D - ROPE_DIM], (128, D - ROPE_DIM)
    ).astype(np.float32)

    inv_freq = (1.0 / (THETA ** (np.arange(HALF, dtype=np.float32) / HALF))).astype(
        np.float32
    )
    in_maps = []
    for c in range(NCORES):
        hT = np.ascontiguousarray(
            h_flat[c * WPC * M : (c + 1) * WPC * M].T
        ).astype(NP_BF16)  # [H, WPC*M]
        hT3 = hT.reshape(KC, 128, WPC * M)  # [kc, p, t]

        blob = np.empty((128, BLOB_COLS), NP_BF16)
        for name, o in BLOB_OFF.items():
            if name == "bias":
                blob[:, o : o + BIAS_COLS] = bias4
            elif name.startswith("wg"):
                k = int(name[2:])
                blob[:, o : o + D] = wg_chunks[k]
            elif name.startswith("wkv"):
                k = int(name[3:])
                blob[:, o : o + D] = wkv_chunks[k]
            else:  # h<g>_<k>
                g, k = name[1:].split("_")
                g, k = int(g), int(k)
                t0 = GOFF[g] * M
                gm = GWS[g] * M
                blob[:, o : o + gm] = hT3[k, :, t0 : t0 + gm]

        t_global = (c % (T // WPC)) * WPC + np.arange(WPC, dtype=np.float32)
        pos = (t_global * M).astype(np.float32)
        freqs = pos[:, None] * inv_freq[None, :]
        cos2 = np.repeat(np.cos(freqs), 2, axis=1).astype(np.float32)
        sin2 = np.repeat(np.sin(freqs), 2, axis=1).astype(np.float32)
        kw = kv_norm_weight
        cos2 = cos2 * kw[None, D - ROPE_DIM : D]
        sinf = np.concatenate(
            [
                -sin2[:, :HALF] * kw[None, D - HALF : D],
                sin2[:, HALF:] * kw[None, D - ROPE_DIM : D - HALF],
            ],
            axis=1,
        )
        cs16 = np.concatenate([cos2, sinf], axis=1)  # [16, 128]
        cs = np.zeros((128, 2 * ROPE_DIM), np.float32)
        cs[:WPC] = cs16
        cswn = np.ascontiguousarray(np.concatenate([cs, wn], axis=1))
        in_maps.append({"blob_in": blob, "cswn_in": cswn})
    return in_maps


def _assemble(results):
    full = np.concatenate([r["out_d"] for r in results], axis=0)  # [128, 128]
    return full.reshape(B, 1, T, D).astype(np.float32)


def _run(inputs, trace=False, **spmd_kwargs):
    nc = _get_nc()
    in_maps = _make_in_maps(
        inputs["hidden_states"],
        inputs["w_kv"],
        inputs["w_gate"],
        inputs["position_bias"],
        inputs["kv_norm_weight"],
    )
    res = run_bass_kernel_spmd(
        nc, in_maps, core_ids=list(range(NCORES)), trace=trace, **spmd_kwargs
    )
    return _assemble(res.results), res


def kernel(
    hidden_states,
    q_residual=None,
    position_ids=None,
    w_kv=None,
    w_gate=None,
    position_bias=None,
    kv_norm_weight=None,
):
    out, _ = _run(
        {
            "hidden_states": hidden_states,
            "w_kv": w_kv,
            "w_gate": w_gate,
            "position_bias": position_bias,
            "kv_norm_weight": kv_norm_weight,
        }
    )
    return out
